# revision 1
# baseline (speedup 1.0000x reference)
"""Trainium2 Bass kernel for KL-divergence attention.

Math used (exactly equivalent to the reference model):
  q = x@Wq, k = x@Wk, v = x@Wv
  kl_ij  = sum_h p_i log p_i - p_i . logq_j   (p = softmax(q), logq = log_softmax(k))
  attn   = softmax_j(-kl_ij) = softmax_j(p_i . logq_j)     [neg-entropy cancels]
         = softmax_j(p_i . k_j - lse_j)
  Using exp(s - lse_j) = exp(s)/sk_j (sk_j = sum_h exp(k_jh)), both the
  numerator (attn @ v) and the softmax denominator absorb the 1/sk_j factor
  into the V rows and the row-sum matmul, so no log/log-softmax is needed:
    e'_ij = exp(p_i . k_j)
    out_i = (sum_j e'_ij * (v_j/sk_j)) / (sum_j e'_ij / sk_j)

Layout strategy (per core, 4 batches of the 32, data-parallel over 8 cores):
  - xT (d on partitions) via PE transposes of the DMA'd x rows
  - q/k/v row blocks [128, 512] via matmuls with xT slices stationary
  - p = exp(q)*rq in natural layout (softmax over free dim, fused row-sum
    via the activation accum_out), then PE-transposed to pT [h, i]
  - k cast to bf16 and PE-transposed to kT [h, j]
  - scores computed directly TRANSPOSED: sT[j, i] = kT.T @ pT so that the
    second matmul needs no transpose of exp(scores):
    out[i, h] = eT.T @ v_scaled, rowsum l[i] = eT.T @ rsk  (rhs = 1/sk col)
  - out = out_unnorm * 1/l (per-partition scalar), DMA back fp32
"""

import numpy as np

import concourse.bass as bass
import concourse.tile as tile
from concourse import bacc, mybir
from concourse.bass_utils import run_bass_kernel_spmd
from concourse.masks import make_identity

B, S, D, H = 32, 2048, 512, 512
NCORES = 8
BPC = B // NCORES  # batches per core
P = 128
NB = S // P   # 16 row blocks per batch
ND = D // P   # 4 contraction chunks
NH = H // P   # 4 h chunks
NG = 4        # phase-3 i groups
GW = S // NG  # 512 i columns per group

FP32 = mybir.dt.float32
BF16 = mybir.dt.bfloat16
EXP = mybir.ActivationFunctionType.Exp


def _emit(tc):
    nc = tc.nc
    x = nc.dram_tensor("x", [BPC, S, D], FP32, kind="ExternalInput").ap()
    wq = nc.dram_tensor("Wq", [D, H], FP32, kind="ExternalInput").ap()
    wk = nc.dram_tensor("Wk", [D, H], FP32, kind="ExternalInput").ap()
    wv = nc.dram_tensor("Wv", [D, H], FP32, kind="ExternalInput").ap()
    out = nc.dram_tensor("out", [BPC, S, H], FP32, kind="ExternalOutput").ap()

    import contextlib

    with contextlib.ExitStack() as ctx:
        consts = ctx.enter_context(tc.tile_pool(name="consts", bufs=1))
        wstage = ctx.enter_context(tc.tile_pool(name="wstage", bufs=1))
        big = ctx.enter_context(tc.tile_pool(name="big", bufs=1))
        vpool = ctx.enter_context(tc.tile_pool(name="vpool", bufs=17))
        epool = ctx.enter_context(tc.tile_pool(name="epool", bufs=18))
        stage = ctx.enter_context(tc.tile_pool(name="stage", bufs=4))
        small = ctx.enter_context(tc.tile_pool(name="small", bufs=4))
        outp = ctx.enter_context(tc.tile_pool(name="outp", bufs=4))
        psS = ctx.enter_context(tc.tile_pool(name="psS", bufs=4, space="PSUM"))
        psA = ctx.enter_context(tc.tile_pool(name="psA", bufs=4, space="PSUM"))

        ident32 = consts.tile([P, P], FP32)
        make_identity(nc, ident32)
        ident16 = consts.tile([P, P], BF16)
        nc.vector.tensor_copy(ident16, ident32)

        # Weights: load fp32, cast to bf16, chunked on the contraction dim.
        w_bf = []
        for w_ap, nm in ((wq, "wq"), (wk, "wk"), (wv, "wv")):
            wt = consts.tile([P, ND, H], BF16, name=f"{nm}_bf")
            wst = wstage.tile([P, ND, H], FP32, tag="wst", name=f"{nm}_st")
            nc.sync.dma_start(out=wst, in_=w_ap.rearrange("(c p) h -> p c h", p=P))
            for c in range(ND):
                nc.vector.tensor_copy(out=wt[:, c, :], in_=wst[:, c, :])
            w_bf.append(wt)
        wq_b, wk_b, wv_b = w_bf

        for b in range(BPC):
            # ---- phase 1: x rows in, xT (bf16, d on partitions) out ----
            xT = [big.tile([P, S], BF16, tag=f"xT{dc}", name=f"xT{dc}_{b}")
                  for dc in range(ND)]
            for ib in range(NB):
                x_st = stage.tile([P, D], FP32, tag="x_st", bufs=6)
                nc.sync.dma_start(out=x_st, in_=x[b, ib * P:(ib + 1) * P, :])
                for dc in range(ND):
                    tp = psS.tile([P, P], FP32, tag="s", name="tpx")
                    nc.tensor.transpose(tp, x_st[:, dc * P:(dc + 1) * P], ident32)
                    nc.any.tensor_copy(out=xT[dc][:, ib * P:(ib + 1) * P], in_=tp)

            # ---- phase 2: projections + softmax pieces ----
            pT = [big.tile([P, S], BF16, tag=f"pT{hc}", name=f"pT{hc}_{b}")
                  for hc in range(NH)]
            kT = [big.tile([P, S], BF16, tag=f"kT{hc}", name=f"kT{hc}_{b}")
                  for hc in range(NH)]
            rsk_all = small.tile([P, NB], BF16, tag="rsk_all", bufs=2)
            v_tiles = []
            p_tiles = {}
            k_tiles = {}

            def emit_tr(jb):
                for hc in range(NH):
                    tpp = psS.tile([P, P], BF16, tag="s", name="tpp")
                    nc.tensor.transpose(
                        tpp, p_tiles[jb][:, hc * P:(hc + 1) * P], ident16)
                    nc.any.tensor_copy(
                        out=pT[hc][:, jb * P:(jb + 1) * P], in_=tpp)
                    tpk = psS.tile([P, P], BF16, tag="s", name="tpk")
                    nc.tensor.transpose(
                        tpk, k_tiles[jb][:, hc * P:(hc + 1) * P], ident16)
                    nc.any.tensor_copy(
                        out=kT[hc][:, jb * P:(jb + 1) * P], in_=tpk)

            for ib in range(NB):
                q_ps = psA.tile([P, H], FP32, tag="a", name="q_ps")
                k_ps = psA.tile([P, H], FP32, tag="a", name="k_ps")
                v_ps = psA.tile([P, H], FP32, tag="a", name="v_ps")
                for ps, wt in ((q_ps, wq_b), (k_ps, wk_b), (v_ps, wv_b)):
                    for dc in range(ND):
                        nc.tensor.matmul(
                            ps, xT[dc][:, ib * P:(ib + 1) * P], wt[:, dc, :],
                            start=(dc == 0), stop=(dc == ND - 1))

                eq_sb = stage.tile([P, H], BF16, tag="eq", bufs=3)
                sq = small.tile([P, 1], FP32, tag="sq")
                nc.scalar.activation(eq_sb, q_ps, EXP, accum_out=sq)
                rq = small.tile([P, 1], FP32, tag="rq")
                nc.vector.reciprocal(rq, sq)
                p_sb = stage.tile([P, H], BF16, tag="p", bufs=4)
                nc.vector.tensor_scalar_mul(p_sb, eq_sb, rq)

                ek_sb = stage.tile([P, H], BF16, tag="ek", bufs=2)
                sk = small.tile([P, 1], FP32, tag="sk")
                nc.scalar.activation(ek_sb, k_ps, EXP, accum_out=sk)
                rsk = small.tile([P, 1], FP32, tag="rsk")
                nc.vector.reciprocal(rsk, sk)
                nc.any.tensor_copy(out=rsk_all[:, ib:ib + 1], in_=rsk)
                v_sb = vpool.tile([P, H], BF16, tag="v")
                nc.vector.tensor_scalar_mul(v_sb, v_ps, rsk)
                k_sb = stage.tile([P, H], BF16, tag="k", bufs=4)
                nc.any.tensor_copy(out=k_sb, in_=k_ps)

                v_tiles.append(v_sb)
                p_tiles[ib] = p_sb
                k_tiles[ib] = k_sb
                if ib >= 2:
                    emit_tr(ib - 2)
            emit_tr(NB - 2)
            emit_tr(NB - 1)

            # ---- phase 3: scores (transposed), exp, attention output ----
            for g in range(NG):
                eT = []
                for jb in range(NB):
                    s_ps = psS.tile([P, GW], FP32, tag="s", name="s_ps")
                    for hc in range(NH):
                        nc.tensor.matmul(
                            s_ps, kT[hc][:, jb * P:(jb + 1) * P],
                            pT[hc][:, g * GW:(g + 1) * GW],
                            start=(hc == 0), stop=(hc == NH - 1))
                    e_sb = epool.tile([P, GW], BF16, tag="e")
                    nc.scalar.activation(e_sb, s_ps, EXP)
                    eT.append(e_sb)
                for il in range(NG):
                    ib = g * NG + il
                    o_ps = psA.tile([P, H], FP32, tag="a", name="o_ps")
                    l_ps = psA.tile([P, 1], FP32, tag="a", name="l_ps")
                    for jc in range(NB):
                        lhs = eT[jc][:, il * P:(il + 1) * P]
                        nc.tensor.matmul(o_ps, lhs, v_tiles[jc],
                                         start=(jc == 0), stop=(jc == NB - 1))
                        nc.tensor.matmul(l_ps, lhs, rsk_all[:, jc:jc + 1],
                                         start=(jc == 0), stop=(jc == NB - 1))
                    rl = small.tile([P, 1], FP32, tag="rl")
                    nc.vector.reciprocal(rl, l_ps)
                    o_sb = outp.tile([P, H], FP32, tag="o")
                    nc.vector.tensor_scalar_mul(o_sb, o_ps, rl)
                    nc.sync.dma_start(
                        out=out[b, ib * P:(ib + 1) * P, :], in_=o_sb)


_NC_CACHE = {}


def _get_nc():
    if "nc" not in _NC_CACHE:
        nc = bacc.Bacc("TRN2", target_bir_lowering=False, debug=False)
        with tile.TileContext(nc) as tc:
            _emit(tc)
        nc.compile()
        _NC_CACHE["nc"] = nc
    return _NC_CACHE["nc"]


def _run(inputs, trace=False, trace_cores=None):
    nc = _get_nc()
    x = np.ascontiguousarray(np.asarray(inputs["x"], dtype=np.float32))
    wq = np.ascontiguousarray(np.asarray(inputs["Wq"], dtype=np.float32))
    wk = np.ascontiguousarray(np.asarray(inputs["Wk"], dtype=np.float32))
    wv = np.ascontiguousarray(np.asarray(inputs["Wv"], dtype=np.float32))
    in_maps = [
        {"x": np.ascontiguousarray(x[c * BPC:(c + 1) * BPC]),
         "Wq": wq, "Wk": wk, "Wv": wv}
        for c in range(NCORES)
    ]
    res = run_bass_kernel_spmd(
        nc, in_maps, core_ids=list(range(NCORES)),
        trace=trace, trace_cores=trace_cores)
    full = np.concatenate([res.results[c]["out"] for c in range(NCORES)], axis=0)
    return full, res


def kernel(**inputs) -> np.ndarray:
    out, _ = _run(inputs)
    return out


# revision 2
# speedup vs baseline: 1.1700x; 1.1700x over previous
"""Trainium2 Bass kernel for KL-divergence attention.

Math used (exactly equivalent to the reference model):
  q = x@Wq, k = x@Wk, v = x@Wv
  kl_ij  = sum_h p_i log p_i - p_i . logq_j   (p = softmax(q), logq = log_softmax(k))
  attn   = softmax_j(-kl_ij) = softmax_j(p_i . logq_j)     [neg-entropy cancels]
         = softmax_j(p_i . k_j - lse_j)
  With exp(s - lse_j) = exp(s)/sk_j (sk_j = sum_h exp(k_jh)), the 1/sk_j
  factor is absorbed into the V rows and the softmax-denominator matmul, so
  no log is needed:
    e'_ij = exp(p_i . k_j)
    out_i = (sum_j e'_ij * (v_j/sk_j)) / (sum_j e'_ij / sk_j)

Layout / precision strategy (per core, 4 of the 32 batches, data-parallel):
  - fp16 operands for all projection / attention-output matmuls (same PE rate
    as bf16, 8x finer mantissa); fp32 PSUM accumulation everywhere
  - the pairwise scores GEMM runs in fp8e4 with perf_mode=DoubleRow
    (K=256 per matmul, 2x ALU): p is pre-scaled by 256 to sit in fp8's
    normal range, undone for free via the activation's scale on exp
  - scores are computed TRANSPOSED (sT[j,i] = kT.T @ pT) so exp(scores)
    feeds the output GEMM as its stationary operand with no transpose
  - all 128x128 transposes are REGULAR matmuls against an identity (or a
    diag(256/sq) for p, folding the softmax normalization into the
    transpose) — transpose-mode does not engage the PE clock-gate and runs
    at ~1.2 GHz; regular matmuls run warm at 2.4 GHz
"""

import numpy as np

import concourse.bass as bass
import concourse.tile as tile
from concourse import bacc, mybir
from concourse.bass_utils import run_bass_kernel_spmd
from concourse.masks import make_identity

B, S, D, H = 32, 2048, 512, 512
NCORES = 8
BPC = B // NCORES  # batches per core
P = 128
NB = S // P   # 16 row blocks per batch
ND = D // P   # 4 contraction chunks
NH = H // P   # 4 h chunks
NG = 4        # i groups in phase 3
GW = S // NG  # 512 i columns per group

FP32 = mybir.dt.float32
FP16 = mybir.dt.float16
FP8 = mybir.dt.float8e4
EXP = mybir.ActivationFunctionType.Exp
DR = mybir.MatmulPerfMode.DoubleRow
PS = 256.0  # p pre-scale so fp8 sees normal-range values


def _emit(tc):
    nc = tc.nc
    x = nc.dram_tensor("x", [BPC, S, D], FP32, kind="ExternalInput").ap()
    wq = nc.dram_tensor("Wq", [D, H], FP32, kind="ExternalInput").ap()
    wk = nc.dram_tensor("Wk", [D, H], FP32, kind="ExternalInput").ap()
    wv = nc.dram_tensor("Wv", [D, H], FP32, kind="ExternalInput").ap()
    out = nc.dram_tensor("out", [BPC, S, H], FP32, kind="ExternalOutput").ap()

    import contextlib

    with contextlib.ExitStack() as ctx:
        consts = ctx.enter_context(tc.tile_pool(name="consts", bufs=1))
        wstage = ctx.enter_context(tc.tile_pool(name="wstage", bufs=1))
        big = ctx.enter_context(tc.tile_pool(name="big", bufs=1))
        vpool = ctx.enter_context(tc.tile_pool(name="vpool", bufs=17))
        epool = ctx.enter_context(tc.tile_pool(name="epool", bufs=34))
        stage = ctx.enter_context(tc.tile_pool(name="stage", bufs=4))
        small = ctx.enter_context(tc.tile_pool(name="small", bufs=4))
        outp = ctx.enter_context(tc.tile_pool(name="outp", bufs=4))
        psS = ctx.enter_context(tc.tile_pool(name="psS", bufs=5, space="PSUM"))
        psA = ctx.enter_context(tc.tile_pool(name="psA", bufs=3, space="PSUM"))

        ident32 = consts.tile([P, P], FP32)
        make_identity(nc, ident32)
        ident16 = consts.tile([P, P], FP16)
        nc.vector.tensor_copy(ident16, ident32)
        ident8 = consts.tile([P, P], FP8)
        nc.vector.tensor_copy(ident8, ident32)

        # Weights: load fp32, cast to fp16, chunked on the contraction dim.
        w_f16 = []
        for w_ap, nm in ((wq, "wq"), (wk, "wk"), (wv, "wv")):
            wt = consts.tile([P, ND, H], FP16, name=f"{nm}_f16")
            wst = wstage.tile([P, ND, H], FP32, tag="wst", name=f"{nm}_st")
            nc.sync.dma_start(out=wst, in_=w_ap.rearrange("(c p) h -> p c h", p=P))
            for c in range(ND):
                nc.vector.tensor_copy(out=wt[:, c, :], in_=wst[:, c, :])
            w_f16.append(wt)
        wq_f, wk_f, wv_f = w_f16

        for b in range(BPC):
            # ---- phase 1: x rows in -> xT (fp16, d on partitions) ----
            xT = [big.tile([P, S], FP16, tag=f"xT{dc}", name=f"xT{dc}_{b}")
                  for dc in range(ND)]
            for ib in range(NB):
                x_st = stage.tile([P, D], FP32, tag="x_st", bufs=6)
                nc.sync.dma_start(out=x_st, in_=x[b, ib * P:(ib + 1) * P, :])
                x16 = stage.tile([P, D], FP16, tag="x16", bufs=4)
                nc.any.tensor_copy(out=x16, in_=x_st)
                for dc in range(ND):
                    tp = psS.tile([P, P], FP32, tag="s", name="tpx")
                    nc.tensor.matmul(tp, x16[:, dc * P:(dc + 1) * P], ident16,
                                     start=True, stop=True)
                    nc.any.tensor_copy(out=xT[dc][:, ib * P:(ib + 1) * P], in_=tp)

            # ---- phase 2: projections, softmax pieces, transposed p/k ----
            pT = big.tile([P, NH, S], FP8, tag="pT", name=f"pT_{b}")
            kT = big.tile([P, NH, S], FP8, tag="kT", name=f"kT_{b}")
            rsk_all = small.tile([P, NB], FP16, tag="rsk_all", bufs=2)
            v_tiles = []
            eq_tiles = {}
            diag_tiles = {}
            k8_tiles = {}

            def emit_tr(jb):
                for hc in range(NH):
                    tpp = psS.tile([P, P], FP32, tag="s", name="tpp")
                    nc.tensor.matmul(
                        tpp, eq_tiles[jb][:, hc * P:(hc + 1) * P], diag_tiles[jb],
                        start=True, stop=True)
                    nc.any.tensor_copy(
                        out=pT[:, hc, jb * P:(jb + 1) * P], in_=tpp)
                    tpk = psS.tile([P, P], FP32, tag="s", name="tpk")
                    nc.tensor.matmul(
                        tpk, k8_tiles[jb][:, hc * P:(hc + 1) * P], ident8,
                        start=True, stop=True)
                    nc.any.tensor_copy(
                        out=kT[:, hc, jb * P:(jb + 1) * P], in_=tpk)

            for ib in range(NB):
                q_ps = psA.tile([P, H], FP32, tag="a", name="q_ps")
                k_ps = psA.tile([P, H], FP32, tag="a", name="k_ps")
                v_ps = psA.tile([P, H], FP32, tag="a", name="v_ps")
                for ps, wt in ((q_ps, wq_f), (k_ps, wk_f), (v_ps, wv_f)):
                    for dc in range(ND):
                        nc.tensor.matmul(
                            ps, xT[dc][:, ib * P:(ib + 1) * P], wt[:, dc, :],
                            start=(dc == 0), stop=(dc == ND - 1))

                eq_sb = stage.tile([P, H], FP16, tag="eq", bufs=4)
                sq = small.tile([P, 1], FP32, tag="sq")
                nc.scalar.activation(eq_sb, q_ps, EXP, accum_out=sq)
                rq = small.tile([P, 1], FP32, tag="rq")
                nc.vector.reciprocal(rq, sq)
                # diag(256/sq): folds p-normalization + fp8 pre-scale into
                # the p transpose matmul
                diag = stage.tile([P, P], FP16, tag="diag", bufs=4)
                nc.vector.tensor_scalar(
                    diag, ident16, rq, PS,
                    op0=mybir.AluOpType.mult, op1=mybir.AluOpType.mult)

                ek_sb = stage.tile([P, H], FP16, tag="ek", bufs=2)
                sk = small.tile([P, 1], FP32, tag="sk")
                nc.scalar.activation(ek_sb, k_ps, EXP, accum_out=sk)
                rsk = small.tile([P, 1], FP32, tag="rsk")
                nc.vector.reciprocal(rsk, sk)
                nc.any.tensor_copy(out=rsk_all[:, ib:ib + 1], in_=rsk)
                v_sb = vpool.tile([P, H], FP16, tag="v")
                nc.vector.tensor_scalar_mul(v_sb, v_ps, rsk)
                k8_sb = stage.tile([P, H], FP8, tag="k8", bufs=4)
                nc.any.tensor_copy(out=k8_sb, in_=k_ps)

                v_tiles.append(v_sb)
                eq_tiles[ib] = eq_sb
                diag_tiles[ib] = diag
                k8_tiles[ib] = k8_sb
                if ib >= 2:
                    emit_tr(ib - 2)
            emit_tr(NB - 2)
            emit_tr(NB - 1)

            # ---- phase 3: fp8 DoubleRow scores (transposed), exp, output ----
            for gp in range(NG // 2):
                igs = (2 * gp, 2 * gp + 1)
                eT = {ig: [] for ig in igs}
                for jb in range(NB):
                    s_ps = {ig: psS.tile([P, GW], FP32, tag="s", name="s_ps")
                            for ig in igs}
                    for pair in range(2):
                        lhs = kT[:, 2 * pair:2 * pair + 2, jb * P:(jb + 1) * P]
                        for ig in igs:
                            nc.tensor.matmul(
                                s_ps[ig], lhs,
                                pT[:, 2 * pair:2 * pair + 2,
                                   ig * GW:(ig + 1) * GW],
                                start=(pair == 0), stop=(pair == 1),
                                perf_mode=DR)
                    for ig in igs:
                        e_sb = epool.tile([P, GW], FP16, tag="e")
                        nc.scalar.activation(e_sb, s_ps[ig], EXP, scale=1.0 / PS)
                        eT[ig].append(e_sb)
                for ig in igs:
                    for il in range(NG):
                        ib = ig * NG + il
                        o_ps = psA.tile([P, H], FP32, tag="a", name="o_ps")
                        l_ps = psA.tile([P, 1], FP32, tag="a", name="l_ps")
                        for jc in range(NB):
                            lhs = eT[ig][jc][:, il * P:(il + 1) * P]
                            nc.tensor.matmul(
                                o_ps, lhs, v_tiles[jc],
                                start=(jc == 0), stop=(jc == NB - 1))
                            nc.tensor.matmul(
                                l_ps, lhs, rsk_all[:, jc:jc + 1],
                                start=(jc == 0), stop=(jc == NB - 1))
                        rl = small.tile([P, 1], FP32, tag="rl")
                        nc.vector.reciprocal(rl, l_ps)
                        o_sb = outp.tile([P, H], FP32, tag="o")
                        nc.vector.tensor_scalar_mul(o_sb, o_ps, rl)
                        nc.sync.dma_start(
                            out=out[b, ib * P:(ib + 1) * P, :], in_=o_sb)


_NC_CACHE = {}


def _get_nc():
    if "nc" not in _NC_CACHE:
        nc = bacc.Bacc("TRN2", target_bir_lowering=False, debug=False)
        with tile.TileContext(nc) as tc:
            _emit(tc)
        nc.compile()
        _NC_CACHE["nc"] = nc
    return _NC_CACHE["nc"]


def _run(inputs, trace=False, trace_cores=None):
    nc = _get_nc()
    x = np.ascontiguousarray(np.asarray(inputs["x"], dtype=np.float32))
    wq = np.ascontiguousarray(np.asarray(inputs["Wq"], dtype=np.float32))
    wk = np.ascontiguousarray(np.asarray(inputs["Wk"], dtype=np.float32))
    wv = np.ascontiguousarray(np.asarray(inputs["Wv"], dtype=np.float32))
    in_maps = [
        {"x": np.ascontiguousarray(x[c * BPC:(c + 1) * BPC]),
         "Wq": wq, "Wk": wk, "Wv": wv}
        for c in range(NCORES)
    ]
    res = run_bass_kernel_spmd(
        nc, in_maps, core_ids=list(range(NCORES)),
        trace=trace, trace_cores=trace_cores)
    full = np.concatenate([res.results[c]["out"] for c in range(NCORES)], axis=0)
    return full, res


def kernel(**inputs) -> np.ndarray:
    out, _ = _run(inputs)
    return out
